# revision 1
# baseline (speedup 1.0000x reference)
"""DotAttention Trainium2 Bass kernel.

out[b] = softmax(Q[b] @ K[b]^T, axis=-1) @ K[b]
  Q: [16, 1024, 4096] f32, K: [16, 2048, 4096] f32 -> out [16, 1024, 4096] f32

Sharding: batch dim across 8 NeuronCores (2 batches/core), fully local.

Per-core pipeline (per batch), all matmuls fp16 with fp32 PSUM accumulation:
  0. Pre-pass: Q/K cast fp32->fp16 by SWDGE DRAM->DRAM DMAs on the (otherwise
     idle) GPSIMD queues into DRAM scratch; batch N+1's pre-pass runs under
     batch N's compute, so only batch 0 pays a cold start.
  1. Stage 1: Q^T and K^T quarter buffers built by xbar DMA-transposes
     reading the fp16 scratch.  At every batch start the critical pair
     (K^T quarter 0 + first Q quarter) goes first with a single xbar mode
     switch; the rest of Q follows as one large transfer under the first
     matmuls.
  2. Logits A = Q K^T per k-quarter (512 keys).  Online softmax: per-quarter
     (negated) local max m_q and sum s_q; e = exp(a - m_q) stored fp16 in
     E[q, k].
  3. Merge pass per q-tile: global max, corrections f_q = exp(m_q - m)
     rescale E (per-partition = per-query), r = 1/sum.
  4. C = E^T.T @ K: E rows xbar-transposed per q-tile; K-natural fp16 chunks
     DMA'd from scratch into the slots vacated by Q^T / K^T quarters.
     Normalization by r folds into the PSUM->SBUF copyback (ACT scale).

SBUF budget (per partition): 64KB Q^T/K-chunks slot + 2x32KB K^T-quarter
slots + 32KB E + ~40KB staging = ~200KB of the ~208KB usable.
"""

import numpy as np

import concourse.bass as bass
import concourse.bacc as bacc
import concourse.mybir as mybir
import concourse.tile as tile
from concourse.bass_utils import run_bass_kernel_spmd

P = 128
N_CORES = 8
B_FULL, LQ, LK, D = 16, 1024, 2048, 4096
B_PER_CORE = B_FULL // N_CORES  # 2

F16 = mybir.dt.float16
F32 = mybir.dt.float32
AX = mybir.AxisListType
AF = mybir.ActivationFunctionType


def build_program(b_per_core=B_PER_CORE, lq=LQ, lk=LK, d=D):
    nqt = lq // P          # q-tiles
    nkc = lk // P          # k-chunks
    nqtr = 4               # k-quarters for online softmax
    kc_per_qtr = nkc // nqtr
    qtr_k = lk // nqtr     # keys per quarter
    dc_n = d // P          # d-chunks
    dh_n = 2               # halves for loads/casts and second-matmul psum
    dhs = d // dh_n

    nc = bacc.Bacc("TRN2", target_bir_lowering=False, debug=False, num_swdge_queues=4)
    q_dram = nc.dram_tensor("query", [b_per_core, lq, d], F32, kind="ExternalInput").ap()
    k_dram = nc.dram_tensor("key", [b_per_core, lk, d], F32, kind="ExternalInput").ap()
    o_dram = nc.dram_tensor("out", [b_per_core, lq, d], F32, kind="ExternalOutput").ap()
    qf16 = nc.dram_tensor("qf16_scratch", [b_per_core, lq, d], F16, kind="Internal").ap()
    kf16 = nc.dram_tensor("kf16_scratch", [b_per_core, lk, d], F16, kind="Internal").ap()

    with tile.TileContext(nc) as tc:
        with (
            # 64KB/partition slot: Q^T during logits, then K-natural chunks 8..15
            tc.tile_pool(name="u64", bufs=1) as u64,
            # 2x 32KB/partition slots: K^T quarters (rotating), then K-natural 0..7
            tc.tile_pool(name="kt32", bufs=2) as kt32,
            # 32KB/partition: unscaled E [q, k] fp16
            tc.tile_pool(name="epool", bufs=1) as epool,
            # 8KB/partition: fp32 staging halves, fp16 row blocks, f32 out staging
            tc.tile_pool(name="s8", bufs=3) as s8,
            # 4KB/partition: E^T tiles for stage C
            tc.tile_pool(name="ettp", bufs=3) as ettp,
            tc.tile_pool(name="stats", bufs=2) as stats,
            tc.tile_pool(name="psum", bufs=2, space="PSUM") as psum,
        ):
            def prepass(b, src, dst, r0, r1):
                """fp32 -> fp16 cast during a SWDGE DRAM->DRAM DMA (GPSIMD
                queues -- parallel to the SP HWDGE ring)."""
                nc.gpsimd.dma_start(out=dst[b, r0:r1, :], in_=src[b, r0:r1, :])

            for b in range(b_per_core):
                # ---- stage 1: Q^T resident [P, dc, q] ----
                qt_full = u64.tile([P, dc_n, lq], F16, tag="u64", name=f"qtf_{b}")
                # critical path at every batch start: K^T quarter 0 and the
                # first Q quarter transpose first (one xbar mode switch), the
                # rest of Q as one big transfer under the first matmuls
                ktq0 = kt32.tile([P, dc_n, qtr_k], F16, tag="k32",
                                 name=f"ktq_{b}_0")
                if b == 0:
                    prepass(b, k_dram, kf16, 0, qtr_k)
                    prepass(b, q_dram, qf16, 0, lq // 4)
                nc.sync.dma_start_transpose(ktq0[:], kf16[b, 0:qtr_k, :])
                nc.sync.dma_start_transpose(
                    qt_full[:, :, 0:lq // 4], qf16[b, 0:lq // 4, :]
                )
                if b == 0:
                    prepass(b, q_dram, qf16, lq // 4, lq)
                nc.sync.dma_start_transpose(
                    qt_full[:, :, lq // 4:], qf16[b, lq // 4:, :]
                )

                # per-batch softmax stats
                M = stats.tile([P, nqt, nqtr], F32, tag="m", name=f"M_{b}")
                S = stats.tile([P, nqt, nqtr], F32, tag="s", name=f"S_{b}")
                F = stats.tile([P, nqt, nqtr], F32, tag="f", name=f"F_{b}")
                R = stats.tile([P, nqt], F32, tag="r", name=f"R_{b}")
                E = epool.tile([P, nqt, lk], F16, tag="e", name=f"E_{b}")

                # ---- logits + per-quarter softmax ----
                for q4 in range(nqtr):
                    if q4 == 0:
                        ktq = ktq0
                    else:
                        if b == 0:
                            prepass(b, k_dram, kf16, q4 * qtr_k,
                                    (q4 + 1) * qtr_k)
                        ktq = kt32.tile([P, dc_n, qtr_k], F16, tag="k32",
                                        name=f"ktq_{b}_{q4}")
                        nc.sync.dma_start_transpose(
                            ktq[:], kf16[b, q4 * qtr_k:(q4 + 1) * qtr_k, :]
                        )
                    for qt in range(nqt):
                        aps = psum.tile([P, qtr_k], F32, tag="ps",
                                        name=f"aps_{b}_{q4}_{qt}")
                        for dc in range(dc_n):
                            nc.tensor.matmul(
                                aps,
                                qt_full[:, dc, qt * P:(qt + 1) * P],
                                ktq[:, dc, :],
                                start=(dc == 0),
                                stop=(dc == dc_n - 1),
                            )
                        nc.vector.reduce_max(
                            M[:, qt, q4:q4 + 1], aps, axis=AX.X, negate=True
                        )
                        nc.scalar.activation(
                            E[:, qt, q4 * qtr_k:(q4 + 1) * qtr_k], aps, AF.Exp,
                            bias=M[:, qt, q4:q4 + 1], scale=1.0,
                            accum_out=S[:, qt, q4:q4 + 1],
                        )

                # ---- next batch's pre-pass overlaps this batch's compute ----
                if b + 1 < b_per_core:
                    prepass(b + 1, q_dram, qf16, 0, lq)
                    for q4 in range(nqtr):
                        prepass(b + 1, k_dram, kf16, q4 * qtr_k, (q4 + 1) * qtr_k)

                # ---- merge pass ----
                for qt in range(nqt):
                    negm = stats.tile([P, 1], F32, tag="negm", name=f"negm_{b}_{qt}")
                    nc.vector.tensor_reduce(
                        negm, M[:, qt, :], axis=AX.X, op=mybir.AluOpType.min
                    )
                    nc.scalar.activation(
                        F[:, qt, :], M[:, qt, :], AF.Exp, bias=negm, scale=-1.0
                    )
                    fs = stats.tile([P, nqtr], F32, tag="fs", name=f"fs_{b}_{qt}")
                    nc.vector.tensor_mul(fs, F[:, qt, :], S[:, qt, :])
                    sg = stats.tile([P, 1], F32, tag="sg", name=f"sg_{b}_{qt}")
                    nc.vector.reduce_sum(sg, fs, axis=AX.X)
                    nc.vector.reciprocal(R[:, qt:qt + 1], sg)
                    for q4 in range(nqtr):
                        sl = E[:, qt, q4 * qtr_k:(q4 + 1) * qtr_k]
                        nc.vector.tensor_scalar_mul(sl, sl, F[:, qt, q4:q4 + 1])

                # ---- second matmul: C = E^T.T @ K ----
                knB0 = kt32.tile([P, kc_per_qtr, d], F16, tag="k32", name=f"knB0_{b}")
                knB1 = kt32.tile([P, kc_per_qtr, d], F16, tag="k32", name=f"knB1_{b}")
                knA = u64.tile([P, nkc - 2 * kc_per_qtr, d], F16, tag="u64",
                               name=f"knA_{b}")

                def kn_chunk(kc):
                    if kc < kc_per_qtr:
                        return knB0[:, kc, :]
                    if kc < 2 * kc_per_qtr:
                        return knB1[:, kc - kc_per_qtr, :]
                    return knA[:, kc - 2 * kc_per_qtr, :]

                for kc in range(nkc):
                    # SWDGE queue: keeps the SP ring free for xposes/stores
                    nc.gpsimd.dma_start(
                        out=kn_chunk(kc), in_=kf16[b, kc * P:(kc + 1) * P, :]
                    )

                for qt in range(nqt):
                    et_t = ettp.tile([P, nkc, P], F16, tag="ett", name=f"ett_{b}_{qt}")
                    nc.sync.dma_start_transpose(et_t, E[:, qt, :])
                    if qt == nqt - 1:
                        # final q-tile: read knB0 (kt32 slot 0) and knA (u64)
                        # early, knB1 last -- frees the slots the next batch's
                        # critical K^T/Q^T transposes need ~10us before this
                        # batch's last matmul retires.  PSUM accumulation
                        # order is associative-free here.
                        korder = (list(range(kc_per_qtr))
                                  + list(range(2 * kc_per_qtr, nkc))
                                  + list(range(kc_per_qtr, 2 * kc_per_qtr)))
                    else:
                        korder = list(range(nkc))
                    for dh in range(dh_n):
                        cps = psum.tile([P, dhs], F32, tag="ps",
                                        name=f"cps_{b}_{qt}_{dh}")
                        for i, kc in enumerate(korder):
                            for nb in range(dhs // 512):
                                nc.tensor.matmul(
                                    cps[:, nb * 512:(nb + 1) * 512],
                                    et_t[:, kc, :],
                                    kn_chunk(kc)[:, dh * dhs + nb * 512:
                                                 dh * dhs + (nb + 1) * 512],
                                    start=(i == 0),
                                    stop=(i == nkc - 1),
                                )
                        c_out = s8.tile([P, dhs], F32, tag="s8", name=f"co_{b}_{qt}_{dh}")
                        nc.scalar.mul(c_out, cps, R[:, qt:qt + 1])
                        nc.sync.dma_start(
                            out=o_dram[b, qt * P:(qt + 1) * P, dh * dhs:(dh + 1) * dhs],
                            in_=c_out,
                        )
    nc.compile()
    return nc


_PROGRAM = None


def _get_program():
    global _PROGRAM
    if _PROGRAM is None:
        _PROGRAM = build_program()
    return _PROGRAM


LAST_RESULTS = None  # BassKernelResults of the most recent kernel() call


def kernel(query: np.ndarray, key: np.ndarray) -> np.ndarray:
    global LAST_RESULTS
    query = np.ascontiguousarray(query, dtype=np.float32)
    key = np.ascontiguousarray(key, dtype=np.float32)
    assert query.shape == (B_FULL, LQ, D), query.shape
    assert key.shape == (B_FULL, LK, D), key.shape

    nc = _get_program()
    in_maps = [
        {
            "query": np.ascontiguousarray(query[i * B_PER_CORE:(i + 1) * B_PER_CORE]),
            "key": np.ascontiguousarray(key[i * B_PER_CORE:(i + 1) * B_PER_CORE]),
        }
        for i in range(N_CORES)
    ]
    res = run_bass_kernel_spmd(nc, in_maps, core_ids=list(range(N_CORES)))
    LAST_RESULTS = res
    out = np.concatenate([r["out"] for r in res.results], axis=0)
    return np.ascontiguousarray(out.astype(np.float32))



# revision 38
# speedup vs baseline: 1.1826x; 1.1826x over previous
"""DotAttention Trainium2 Bass kernel (fp8 DoubleRow edition).

out[b] = softmax(Q[b] @ K[b]^T, axis=-1) @ K[b]
  Q: [16, 1024, 4096] f32, K: [16, 2048, 4096] f32 -> out [16, 1024, 4096] f32

Sharding: batch dim across 8 NeuronCores (2 batches/core), fully local.

All matmuls run in fp8e4 (e4m3) DoubleRow mode: each instruction contracts
256 (two 128-blocks via the paired slot dim) at 0.5 cycles/row -- 4x the
fp16 streaming rate in the cost model.  Precision is held with error
feedback (hi+lo fp8 splits, lo = x - fp8(x) requantized):

  mm1 logits: A ~= Q8 K8^T + Qlo K8^T + Q8 Klo^T   (3 DR terms, f16-sourced)
  mm2:        C ~= E8 K8n + E8 Klon                (2 DR terms)

E is kept in fp16 (exp output), rescaled by the online-softmax correction
in fp16, transposed per q-tile by the xbar (2-byte only), then cast to
fp8 on ACT right before mm2.  K gets two independent splits: transposed
(for mm1, from the fp16 xbar transposes, ACT cast + DVE sub) and natural
(for mm2, hi via SWDGE fp16->fp8 casting DMA straight from the fp16
scratch, lo via DVE sub against streamed fp16 rows).

Online softmax over 5 key sections (256,256,512,512,512): the small lead
sections shorten the batch-0 cold start to the first matmul.

CPU-validated end-to-end numeric flow: rel err ~0.012 (tolerance 2e-2).
"""

import numpy as np

import concourse.bass as bass
import concourse.bacc as bacc
import concourse.mybir as mybir
import concourse.tile as tile
from concourse.bass_utils import run_bass_kernel_spmd

P = 128
N_CORES = 8
B_FULL, LQ, LK, D = 16, 1024, 2048, 4096
B_PER_CORE = B_FULL // N_CORES  # 2
DC = D // P                     # 32 d-chunks
NDP = DC // 2                   # 16 DoubleRow d-chunk pairs
NQT = LQ // P                   # 8 q-tiles
# online-softmax key sections
SEC = 256
NSEC = LK // SEC                # 8 sections
NKP = LK // 256                 # 8 key chunk-pairs for mm2
DH = 2
DHS = D // DH                   # 2048

F16 = mybir.dt.float16
F32 = mybir.dt.float32
F8 = mybir.dt.float8e4
AX = mybir.AxisListType
AF = mybir.ActivationFunctionType
DR = mybir.MatmulPerfMode.DoubleRow


def build_program(b_per_core=B_PER_CORE, lq=LQ, lk=LK, d=D):
    nc = bacc.Bacc("TRN2", target_bir_lowering=False, debug=False, num_swdge_queues=4)
    q_dram = nc.dram_tensor("query", [b_per_core, lq, d], F32, kind="ExternalInput").ap()
    k_dram = nc.dram_tensor("key", [b_per_core, lk, d], F32, kind="ExternalInput").ap()
    o_dram = nc.dram_tensor("out", [b_per_core, lq, d], F16, kind="ExternalOutput").ap()
    qf16 = nc.dram_tensor("qf16_scratch", [b_per_core, lq, d], F16, kind="Internal").ap()
    kf16 = nc.dram_tensor("kf16_scratch", [b_per_core, lk, d], F16, kind="Internal").ap()

    NQB = 4          # q-blocks of 2 q-tiles; mm1 runs a (section, q-block)
    NSTEP = NSEC + NQB - 1  # diagonal wavefront: block (si, qb) at step si+qb

    with tile.TileContext(nc) as tc:
        with (
            # 4x 16KB: mm1 Q^T hi+lo per q-block; mm2 Kn chunk-pairs 4..7
            tc.tile_pool(name="u16", bufs=4) as u16,
            # 5x 16KB: K^T section tiles (rotating, 2-step write slack);
            # mm2 Kn chunk-pairs 0..3
            tc.tile_pool(name="kq16", bufs=5) as kq16,
            # 16KB: E [P, NQT, LK] fp8 (exp writes e4m3 directly)
            tc.tile_pool(name="e16", bufs=1) as e16,
            # 8KB staging: f16 transpose pieces / f16 natural-K halves
            tc.tile_pool(name="stg", bufs=4) as stg,
            tc.tile_pool(name="et8p", bufs=2) as et8p,
            tc.tile_pool(name="outp", bufs=2) as outp,
            tc.tile_pool(name="stats", bufs=2) as stats,
            tc.tile_pool(name="psum", bufs=2, space="PSUM") as psum,
            tc.tile_pool(name="etps", bufs=2, space="PSUM") as etps,
        ):
            def prepass(b, src, dst, r0, r1):
                """fp32 -> fp16 cast via SWDGE DRAM->DRAM DMA (gpsimd queues)."""
                nc.gpsimd.dma_start(out=dst[b, r0:r1, :], in_=src[b, r0:r1, :])

            from concourse.masks import make_identity

            ident = None
            staged = {}

            for b in range(b_per_core):
                if b == 0:
                    prepass(b, q_dram, qf16, 0, 256)
                    prepass(b, k_dram, kf16, 0, 256)

                # Q^T hi+lo, one tile per q-block.  Slot of q-block c is
                # reused by mm2 chunk-pair 4+c; q-block c's last mm1 reader
                # is step 7+c, so the late K chunks stream in during the
                # wavefront tail instead of after mm1.
                qblks = [
                    u16.tile([P, 2, DC, 2 * P], F8, tag="u16", name=f"qb_{b}_{j}")
                    for j in range(NQB)
                ]
                knCs = [
                    u16.tile([P, 2, 2, d], F8, tag="u16", name=f"knC_{b}_{c}")
                    for c in range(4)
                ]
                # kq16: kt_si -> slot si%4; kn_j -> slot of kt_{4+j} (freed
                # after step 7+j).
                kts = [
                    kq16.tile([P, 2, DC, SEC], F8, tag="k16", name=f"kt_{b}_{si}")
                    for si in range(NSEC)
                ]
                kns = [
                    kq16.tile([P, 2, 2, d], F8, tag="k16", name=f"kn_{b}_{c}")
                    for c in range(4)
                ]

                # staged transpose pieces carried across the batch
                # boundary (filled during the previous batch's mm2)
                def qpath_dma(j):
                    r0 = j * P
                    s = stg.tile([P, DC, P], F16, tag="stg", name=f"qstg_{b}_{j}")
                    nc.sync.dma_start_transpose(s, qf16[b, r0:r0 + P, :])
                    return s

                def qpath_fin(j, s):
                    qb = qblks[j // 2]
                    c0 = (j % 2) * P
                    nc.scalar.copy(out=qb[:, 0, :, c0:c0 + P], in_=s)
                    nc.vector.tensor_sub(
                        out=qb[:, 1, :, c0:c0 + P], in0=s, in1=qb[:, 0, :, c0:c0 + P],
                    )

                def qpath(j):
                    """Transpose+split one 128-row piece of Q (q-tile j).
                    All subs stay on DVE: a slow Pool sub at the head of the
                    in-order Pool queue would block SWDGE descriptor
                    generation for every casting DMA behind it."""
                    qpath_fin(j, staged.pop(("q", j), None) or qpath_dma(j))

                def kpath_dma(si, j):
                    r0 = si * SEC + j * P
                    s = stg.tile([P, DC, P], F16, tag="stg", name=f"kstg_{b}_{r0}")
                    nc.sync.dma_start_transpose(s, kf16[b, r0:r0 + P, :])
                    return s

                def kpath_fin(si, j, s):
                    kt = kts[si]
                    c0 = j * P
                    nc.scalar.copy(out=kt[:, 0, :, c0:c0 + P], in_=s)
                    nc.vector.tensor_sub(
                        out=kt[:, 1, :, c0:c0 + P], in0=s, in1=kt[:, 0, :, c0:c0 + P],
                    )

                def kpath(si, j):
                    """Transpose+split one 128-key piece of K section si."""
                    kpath_fin(si, j, staged.pop(("k", si, j), None) or kpath_dma(si, j))

                def kn_tile(c):
                    return kns[c] if c < 4 else knCs[c - 4]

                def kn_dma(c, h):
                    """One d-half of mm2 K chunk-pair c: f16 rows to staging +
                    hi tensor via casting SWDGE DMA (block-paired rows)."""
                    kn = kn_tile(c)
                    src = kf16[b, 256 * c:256 * (c + 1),
                               h * DHS:(h + 1) * DHS].rearrange(
                        "(two p) d -> p two d", two=2)
                    s = stg.tile([P, 2, DHS], F16, tag="stg",
                                 name=f"knstg_{b}_{c}_{h}")
                    nc.sync.dma_start(out=s, in_=src)
                    nc.gpsimd.dma_start(
                        out=kn[:, 0, :, h * DHS:(h + 1) * DHS], in_=src)
                    return s

                def kn_sub(c, h, q, s, eng):
                    """DVE/Pool lo = f16 - hi for one d-quarter piece."""
                    d0 = h * DHS + q * (DHS // 2)
                    kn = kn_tile(c)
                    eng.tensor_sub(
                        out=kn[:, 1, :, d0:d0 + DHS // 2],
                        in0=s[:, :, q * (DHS // 2):(q + 1) * (DHS // 2)],
                        in1=kn[:, 0, :, d0:d0 + DHS // 2],
                    )

                # stats
                M = stats.tile([P, NQT, NSEC], F32, tag="m", name=f"M_{b}")
                S = stats.tile([P, NQT, NSEC], F32, tag="s", name=f"S_{b}")
                F = stats.tile([P, NQT, NSEC], F32, tag="f", name=f"F_{b}")
                R = stats.tile([P, NQT], F32, tag="r", name=f"R_{b}")
                E = e16.tile([P, NQT, lk], F8, tag="e", name=f"E_{b}")

                # ---- stage-1 lead-in: step-0 operands ----
                kpath(0, 0)
                qpath(0)
                if ident is None:
                    ident = stats.tile([P, P], F8, tag="ident", name="ident")
                    make_identity(nc, ident)
                kpath(0, 1)
                qpath(1)
                if b == 0:
                    prepass(b, q_dram, qf16, 256, 512)
                    prepass(b, k_dram, kf16, 256, 512)
                kpath(1, 0)
                if b == 0:
                    prepass(b, k_dram, kf16, 512, 768)
                kpath(1, 1)
                qpath(2)
                qpath(3)

                def mm1_tile(si, qt):
                    kt = kts[si]
                    k0 = si * SEC
                    aps = psum.tile([P, SEC], F32, tag="ps",
                                    name=f"aps_{b}_{si}_{qt}")
                    qb = qblks[qt // 2]
                    qsl = slice((qt % 2) * P, (qt % 2) * P + P)
                    for term, (ti, tk) in enumerate(((0, 0), (1, 0), (0, 1))):
                        for i in range(NDP):
                            nc.tensor.matmul(
                                aps,
                                qb[:, ti, 2 * i:2 * i + 2, qsl],
                                kt[:, tk, 2 * i:2 * i + 2, :],
                                start=(term == 0 and i == 0),
                                stop=(term == 2 and i == NDP - 1),
                                perf_mode=DR,
                            )
                    nc.vector.reduce_max(
                        M[:, qt, si:si + 1], aps, axis=AX.X, negate=True
                    )
                    nc.scalar.activation(
                        E[:, qt, k0:k0 + SEC], aps, AF.Exp,
                        bias=M[:, qt, si:si + 1], scale=1.0,
                        accum_out=S[:, qt, si:si + 1],
                    )

                def merge_et(qt):
                    """Softmax merge + E^T transpose/cast for one q-tile."""
                    negm = stats.tile([P, 1], F32, tag="negm", name=f"negm_{b}_{qt}")
                    nc.vector.tensor_reduce(
                        negm, M[:, qt, :], axis=AX.X, op=mybir.AluOpType.min
                    )
                    nc.scalar.activation(
                        F[:, qt, :], M[:, qt, :], AF.Exp, bias=negm, scale=-1.0
                    )
                    fs = stats.tile([P, NSEC], F32, tag="fs", name=f"fs_{b}_{qt}")
                    nc.vector.tensor_mul(fs, F[:, qt, :], S[:, qt, :])
                    sg = stats.tile([P, 1], F32, tag="sg", name=f"sg_{b}_{qt}")
                    nc.vector.reduce_sum(sg, fs, axis=AX.X)
                    nc.vector.reciprocal(R[:, qt:qt + 1], sg)
                    for si in range(NSEC):
                        sl = E[:, qt, si * SEC:(si + 1) * SEC]
                        nc.vector.tensor_scalar_mul(sl, sl, F[:, qt, si:si + 1])

                    # E^T via PE-array transposes (fp8), 8 blocks per PSUM
                    # bank, one ACT copy per bank
                    et8 = et8p.tile([P, LK // P, P], F8, tag="et8", name=f"et8_{b}_{qt}")
                    for half in range(2):
                        # fp8 PE transposes must write PSUM with element
                        # step 2; the ACT copy compacts the stride
                        ep = etps.tile([P, 8, P, 2], F8, tag="etps",
                                       name=f"etps_{b}_{qt}_{half}")
                        # one accumulation group per bank: start only on
                        # the first block so the bank's zero-region isn't
                        # re-armed over already-written blocks
                        for kc in range(8):
                            kcg = half * 8 + kc
                            nc.tensor.matmul(
                                ep[:, kc, :, 0], E[:, qt, kcg * P:(kcg + 1) * P],
                                ident, is_transpose=True,
                                start=(kc == 0), stop=(kc == 7),
                                skip_group_check=True,
                            )
                        nc.scalar.copy(out=et8[:, half * 8:(half + 1) * 8, :],
                                       in_=ep[:, :, :, 0])
                    return et8

                # ---- mm1 diagonal wavefront with woven supply ----
                kn_pending = []
                supply = []
                et8s = [None] * NQT
                for s in range(NSTEP):
                    blocks = [(si, s - si) for si in range(NSEC)
                              if 0 <= s - si < NQB]
                    # supply for step s+2 (and the tail Kn chunks), one
                    # piece between tiles
                    # Ordering rule: DRAM write->read deps come from
                    # program order, so a prepass piece must be issued
                    # before any transpose that reads it.  ppq(s+2) feeds
                    # this step's qp -> goes first; pp(K, s+3) feeds the
                    # NEXT step's kp -> rides after this step's kp/qp so
                    # transposes aren't queued behind it.
                    weave = []
                    if b == 0 and s + 2 < NQB:
                        r0 = (s + 2) * 256
                        weave.append(("ppq", r0))
                        weave.append(("ppq", r0 + 128))
                    if s + 2 < NSEC:
                        weave.append(("kp", s + 2, 0))
                        weave.append(("kp", s + 2, 1))
                    if s + 2 < NQB:
                        weave.append(("qp", 2 * (s + 2)))
                        weave.append(("qp", 2 * (s + 2) + 1))
                    if b == 0:
                        r0 = (s + 3) * 256
                        if r0 < lk:
                            weave.append(("pp", r0))
                            weave.append(("pp", r0 + 128))
                    if b == 1 and s < 4:
                        # second half of this batch's K prepass rides its own
                        # early steps (first half ran under b0's mm2)
                        r0 = (s + 4) * 256
                        weave.append(("pp", r0))
                        weave.append(("pp", r0 + 128))
                    if 6 <= s <= 9:
                        # kq16 kn chunk s-6 (slot of kt_{s-3}, freed after
                        # step s-3+3 = s)
                        for h in range(DH):
                            weave.append(("kn", s - 6, h))
                    if 7 <= s <= 10:
                        # u16 knC chunk 4+(s-7) (q-block s-7's slot)
                        for h in range(DH):
                            weave.append(("kn", s - 3, h))
                    supply.extend(weave)

                    def issue(op):
                        if op[0] == "kp":
                            kpath(op[1], op[2])
                        elif op[0] == "qp":
                            qpath(op[1])
                        elif op[0] == "pp":
                            prepass(b, k_dram, kf16, op[1], op[1] + 128)
                        elif op[0] == "ppq":
                            prepass(b, q_dram, qf16, op[1], op[1] + 128)
                        elif op[0] == "kn":
                            st = kn_dma(op[1], op[2])
                            kn_pending.append((op[1], op[2], 0, st, nc.vector))
                            kn_pending.append((op[1], op[2], 1, st, nc.vector))

                    tiles = [(si, 2 * qb + t) for si, qb in blocks
                             for t in range(2)]
                    for i, (si, qt) in enumerate(tiles):
                        mm1_tile(si, qt)
                        if s >= 7 and kn_pending and i % 2 == 0:
                            # drain kn lo-subs with priority in the tail so
                            # they don't pile up at the mm1->mm2 transition
                            kn_sub(*kn_pending.pop(0))
                        elif supply:
                            issue(supply.pop(0))
                        elif kn_pending:
                            kn_sub(*kn_pending.pop(0))
                    # flush the step's remaining supply in issue order --
                    # deferring a prepass past a transpose that reads it
                    # would break the program-order write->read dependency
                    while supply:
                        issue(supply.pop(0))
                    # early merges: q-block 0 has all its sections done
                    # after step 7, block 1 after step 8 -- build their E^T
                    # during the wavefront tail so mm2 is gated only by Kn
                    if s == NSTEP - 3:
                        et8s[0] = merge_et(0)
                    elif s == NSTEP - 2:
                        et8s[1] = merge_et(1)

                # mm1 -> mm2 transition: leftover kn subs, then early
                # merges (qt0/qt1 merged in the mm1 tail weave below)
                for args in kn_pending:
                    kn_sub(*args)
                kn_pending = []
                if et8s[0] is None:
                    et8s[0] = merge_et(0)
                if et8s[1] is None:
                    et8s[1] = merge_et(1)

                # mm2 psum-group chunk order: by supply readiness
                CORDER = (0, 4, 1, 5, 2, 6, 3, 7)

                # ---- mm2 per q-tile (merges interleaved) ----
                for qt in range(NQT):
                    et8 = et8s[qt]
                    for dq in range(4):
                        DQ = d // 4
                        cps = psum.tile([P, DQ], F32, tag="ps",
                                        name=f"cps_{b}_{qt}_{dq}")
                        for j in range(DQ // 512):
                            dsl = slice(dq * DQ + j * 512, dq * DQ + (j + 1) * 512)
                            for ci, c in enumerate(CORDER):
                                for t in range(2):
                                    nc.tensor.matmul(
                                        cps[:, j * 512:(j + 1) * 512],
                                        et8[:, 2 * c:2 * c + 2, :],
                                        kn_tile(c)[:, t, :, dsl],
                                        start=(ci == 0 and t == 0),
                                        stop=(ci == NKP - 1 and t == 1),
                                        perf_mode=DR,
                                    )
                        ot = outp.tile([P, DQ], F16, tag="out",
                                       name=f"ot_{b}_{qt}_{dq}")
                        nc.scalar.mul(ot, cps, R[:, qt:qt + 1])
                        nc.sync.dma_start(
                            out=o_dram[b, qt * P:(qt + 1) * P,
                                       dq * DQ:(dq + 1) * DQ],
                            in_=ot,
                        )
                        # next batch's Q + half-K f32->f16 casts ride mm2's
                        # DMA slack (other K half rides b+1's own early mm1)
                        if b + 1 < b_per_core:
                            pi = qt * 4 + dq
                            if pi < 8:
                                prepass(b + 1, q_dram, qf16, pi * 128,
                                        (pi + 1) * 128)
                            elif pi < 16:
                                r0 = (pi - 8) * 128
                                prepass(b + 1, k_dram, kf16, r0, r0 + 128)
                    if qt + 2 < NQT:
                        et8s[qt + 2] = merge_et(qt + 2)
                    if b + 1 < b_per_core and qt == 5:
                        # boundary pre-stage: next batch's first transpose
                        # pieces ride this batch's mm2 tail (stg is idle);
                        # only cast+sub remain at the batch boundary
                        bn = b + 1
                        staged[("q", 0)] = stg.tile(
                            [P, DC, P], F16, tag="stg", name=f"qstg_{bn}_0")
                        nc.sync.dma_start_transpose(
                            staged[("q", 0)], qf16[bn, 0:P, :])
                        staged[("k", 0, 0)] = stg.tile(
                            [P, DC, P], F16, tag="stg", name=f"kstg_{bn}_0")
                        nc.sync.dma_start_transpose(
                            staged[("k", 0, 0)], kf16[bn, 0:P, :])
                    if b + 1 < b_per_core and qt == 6:
                        bn = b + 1
                        staged[("q", 1)] = stg.tile(
                            [P, DC, P], F16, tag="stg", name=f"qstg_{bn}_1")
                        nc.sync.dma_start_transpose(
                            staged[("q", 1)], qf16[bn, P:2 * P, :])
                        staged[("k", 0, 1)] = stg.tile(
                            [P, DC, P], F16, tag="stg", name=f"kstg_{bn}_1")
                        nc.sync.dma_start_transpose(
                            staged[("k", 0, 1)], kf16[bn, P:2 * P, :])
    nc.compile()
    return nc


_PROGRAM = None


def _get_program():
    global _PROGRAM
    if _PROGRAM is None:
        _PROGRAM = build_program()
    return _PROGRAM


LAST_RESULTS = None  # BassKernelResults of the most recent kernel() call


def kernel(query: np.ndarray, key: np.ndarray) -> np.ndarray:
    global LAST_RESULTS
    query = np.ascontiguousarray(query, dtype=np.float32)
    key = np.ascontiguousarray(key, dtype=np.float32)
    assert query.shape == (B_FULL, LQ, D), query.shape
    assert key.shape == (B_FULL, LK, D), key.shape

    nc = _get_program()
    in_maps = [
        {
            "query": np.ascontiguousarray(query[i * B_PER_CORE:(i + 1) * B_PER_CORE]),
            "key": np.ascontiguousarray(key[i * B_PER_CORE:(i + 1) * B_PER_CORE]),
        }
        for i in range(N_CORES)
    ]
    res = run_bass_kernel_spmd(nc, in_maps, core_ids=list(range(N_CORES)))
    LAST_RESULTS = res
    out = np.concatenate([r["out"] for r in res.results], axis=0)
    return np.ascontiguousarray(out.astype(np.float32))


# revision 45
# speedup vs baseline: 1.2160x; 1.0282x over previous
"""DotAttention Trainium2 Bass kernel (fp8 DoubleRow edition).

out[b] = softmax(Q[b] @ K[b]^T, axis=-1) @ K[b]
  Q: [16, 1024, 4096] f32, K: [16, 2048, 4096] f32 -> out [16, 1024, 4096] f32

Sharding: batch dim across 8 NeuronCores (2 batches/core), fully local.

All matmuls run in fp8e4 (e4m3) DoubleRow mode: each instruction contracts
256 (two 128-blocks via the paired slot dim) at 0.5 cycles/row -- 4x the
fp16 streaming rate in the cost model.  Precision is held with error
feedback (hi+lo fp8 splits, lo = x - fp8(x) requantized):

  mm1 logits: A ~= Q8 K8^T + Qlo K8^T + Q8 Klo^T   (3 DR terms, f16-sourced)
  mm2:        C ~= E8 K8n + E8 Klon                (2 DR terms)

E is kept in fp16 (exp output), rescaled by the online-softmax correction
in fp16, transposed per q-tile by the xbar (2-byte only), then cast to
fp8 on ACT right before mm2.  K gets two independent splits: transposed
(for mm1, from the fp16 xbar transposes, ACT cast + DVE sub) and natural
(for mm2, hi via SWDGE fp16->fp8 casting DMA straight from the fp16
scratch, lo via DVE sub against streamed fp16 rows).

Online softmax over 5 key sections (256,256,512,512,512): the small lead
sections shorten the batch-0 cold start to the first matmul.

CPU-validated end-to-end numeric flow: rel err ~0.012 (tolerance 2e-2).
"""

import numpy as np

import concourse.bass as bass
import concourse.bacc as bacc
import concourse.mybir as mybir
import concourse.tile as tile
from concourse.bass_utils import run_bass_kernel_spmd

P = 128
N_CORES = 8
B_FULL, LQ, LK, D = 16, 1024, 2048, 4096
B_PER_CORE = B_FULL // N_CORES  # 2
DC = D // P                     # 32 d-chunks
NDP = DC // 2                   # 16 DoubleRow d-chunk pairs
NQT = LQ // P                   # 8 q-tiles
# online-softmax key sections
SEC = 256
NSEC = LK // SEC                # 8 sections
NKP = LK // 256                 # 8 key chunk-pairs for mm2
DH = 2
DHS = D // DH                   # 2048

F16 = mybir.dt.float16
F32 = mybir.dt.float32
F8 = mybir.dt.float8e4
AX = mybir.AxisListType
AF = mybir.ActivationFunctionType
DR = mybir.MatmulPerfMode.DoubleRow


def build_program(b_per_core=B_PER_CORE, lq=LQ, lk=LK, d=D):
    nc = bacc.Bacc("TRN2", target_bir_lowering=False, debug=False, num_swdge_queues=4)
    q_dram = nc.dram_tensor("query", [b_per_core, lq, d], F32, kind="ExternalInput").ap()
    k_dram = nc.dram_tensor("key", [b_per_core, lk, d], F32, kind="ExternalInput").ap()
    o_dram = nc.dram_tensor("out", [b_per_core, lq, d], F16, kind="ExternalOutput").ap()
    qf16 = nc.dram_tensor("qf16_scratch", [b_per_core, lq, d], F16, kind="Internal").ap()
    kf16 = nc.dram_tensor("kf16_scratch", [b_per_core, lk, d], F16, kind="Internal").ap()

    NQB = 4          # q-blocks of 2 q-tiles; mm1 runs a (section, q-block)
    NSTEP = NSEC + NQB - 1  # diagonal wavefront: block (si, qb) at step si+qb

    with tile.TileContext(nc) as tc:
        with (
            # 4x 16KB: mm1 Q^T hi+lo per q-block; mm2 Kn chunk-pairs 4..7
            tc.tile_pool(name="u16", bufs=4) as u16,
            # 5x 16KB: K^T section tiles (rotating, 2-step write slack);
            # mm2 Kn chunk-pairs 0..3
            tc.tile_pool(name="kq16", bufs=5) as kq16,
            # 16KB: E [P, NQT, LK] fp8 (exp writes e4m3 directly)
            tc.tile_pool(name="e16", bufs=1) as e16,
            # 8KB staging: f16 transpose pieces / f16 natural-K halves
            tc.tile_pool(name="stg", bufs=4) as stg,
            tc.tile_pool(name="et8p", bufs=3) as et8p,
            tc.tile_pool(name="outp", bufs=3) as outp,
            tc.tile_pool(name="stats", bufs=2) as stats,
            tc.tile_pool(name="psum", bufs=3, space="PSUM") as psum,
            tc.tile_pool(name="etps", bufs=2, space="PSUM") as etps,
        ):
            def prepass(b, src, dst, r0, r1):
                """fp32 -> fp16 cast via SWDGE DRAM->DRAM DMA (gpsimd queues)."""
                nc.gpsimd.dma_start(out=dst[b, r0:r1, :], in_=src[b, r0:r1, :])

            from concourse.masks import make_identity

            ident = None
            staged = {}

            for b in range(b_per_core):
                if b == 0:
                    prepass(b, q_dram, qf16, 0, 256)
                    prepass(b, k_dram, kf16, 0, 256)

                # Q^T hi+lo, one tile per q-block.  Slot of q-block c is
                # reused by mm2 chunk-pair 4+c; q-block c's last mm1 reader
                # is step 7+c, so the late K chunks stream in during the
                # wavefront tail instead of after mm1.
                qblks = [
                    u16.tile([P, 2, DC, 2 * P], F8, tag="u16", name=f"qb_{b}_{j}")
                    for j in range(NQB)
                ]
                knCs = [
                    u16.tile([P, 2, 2, d], F8, tag="u16", name=f"knC_{b}_{c}")
                    for c in range(4)
                ]
                # kq16: kt_si -> slot si%4; kn_j -> slot of kt_{4+j} (freed
                # after step 7+j).
                kts = [
                    kq16.tile([P, 2, DC, SEC], F8, tag="k16", name=f"kt_{b}_{si}")
                    for si in range(NSEC)
                ]
                kns = [
                    kq16.tile([P, 2, 2, d], F8, tag="k16", name=f"kn_{b}_{c}")
                    for c in range(4)
                ]

                # staged transpose pieces carried across the batch
                # boundary (filled during the previous batch's mm2)
                def qpath_dma(j):
                    r0 = j * P
                    s = stg.tile([P, DC, P], F16, tag="stg", name=f"qstg_{b}_{j}")
                    nc.sync.dma_start_transpose(s, qf16[b, r0:r0 + P, :])
                    return s

                def qpath_fin(j, s):
                    qb = qblks[j // 2]
                    c0 = (j % 2) * P
                    nc.scalar.copy(out=qb[:, 0, :, c0:c0 + P], in_=s)
                    nc.vector.tensor_sub(
                        out=qb[:, 1, :, c0:c0 + P], in0=s, in1=qb[:, 0, :, c0:c0 + P],
                    )

                def qpath(j):
                    """Transpose+split one 128-row piece of Q (q-tile j).
                    All subs stay on DVE: a slow Pool sub at the head of the
                    in-order Pool queue would block SWDGE descriptor
                    generation for every casting DMA behind it."""
                    qpath_fin(j, staged.pop(("q", j), None) or qpath_dma(j))

                def kpath_dma(si, j):
                    r0 = si * SEC + j * P
                    s = stg.tile([P, DC, P], F16, tag="stg", name=f"kstg_{b}_{r0}")
                    nc.sync.dma_start_transpose(s, kf16[b, r0:r0 + P, :])
                    return s

                def kpath_fin(si, j, s):
                    kt = kts[si]
                    c0 = j * P
                    nc.scalar.copy(out=kt[:, 0, :, c0:c0 + P], in_=s)
                    nc.vector.tensor_sub(
                        out=kt[:, 1, :, c0:c0 + P], in0=s, in1=kt[:, 0, :, c0:c0 + P],
                    )

                def kpath(si, j):
                    """Transpose+split one 128-key piece of K section si."""
                    kpath_fin(si, j, staged.pop(("k", si, j), None) or kpath_dma(si, j))

                def kn_tile(c):
                    return kns[c] if c < 4 else knCs[c - 4]

                def kn_dma(c, h):
                    """One d-half of mm2 K chunk-pair c: f16 rows to staging +
                    hi tensor via casting SWDGE DMA (block-paired rows)."""
                    kn = kn_tile(c)
                    src = kf16[b, 256 * c:256 * (c + 1),
                               h * DHS:(h + 1) * DHS].rearrange(
                        "(two p) d -> p two d", two=2)
                    s = stg.tile([P, 2, DHS], F16, tag="stg",
                                 name=f"knstg_{b}_{c}_{h}")
                    nc.sync.dma_start(out=s, in_=src)
                    nc.gpsimd.dma_start(
                        out=kn[:, 0, :, h * DHS:(h + 1) * DHS], in_=src)
                    return s

                def kn_sub(c, h, q, s, eng):
                    """DVE/Pool lo = f16 - hi for one d-quarter piece."""
                    d0 = h * DHS + q * (DHS // 2)
                    kn = kn_tile(c)
                    eng.tensor_sub(
                        out=kn[:, 1, :, d0:d0 + DHS // 2],
                        in0=s[:, :, q * (DHS // 2):(q + 1) * (DHS // 2)],
                        in1=kn[:, 0, :, d0:d0 + DHS // 2],
                    )

                # stats
                M = stats.tile([P, NQT, NSEC], F32, tag="m", name=f"M_{b}")
                S = stats.tile([P, NQT, NSEC], F32, tag="s", name=f"S_{b}")
                F = stats.tile([P, NQT, NSEC], F32, tag="f", name=f"F_{b}")
                R = stats.tile([P, NQT], F32, tag="r", name=f"R_{b}")
                E = e16.tile([P, NQT, lk], F8, tag="e", name=f"E_{b}")

                # ---- stage-1 lead-in: step-0 operands ----
                kpath(0, 0)
                qpath(0)
                if ident is None:
                    ident = stats.tile([P, P], F8, tag="ident", name="ident")
                    make_identity(nc, ident)
                kpath(0, 1)
                qpath(1)
                if b == 0:
                    prepass(b, q_dram, qf16, 256, 512)
                    prepass(b, k_dram, kf16, 256, 512)
                kpath(1, 0)
                if b == 0:
                    prepass(b, k_dram, kf16, 512, 768)
                kpath(1, 1)
                qpath(2)
                qpath(3)

                def mm1_tile(si, qt):
                    kt = kts[si]
                    k0 = si * SEC
                    aps = psum.tile([P, SEC], F32, tag="ps",
                                    name=f"aps_{b}_{si}_{qt}")
                    qb = qblks[qt // 2]
                    qsl = slice((qt % 2) * P, (qt % 2) * P + P)
                    for term, (ti, tk) in enumerate(((0, 0), (1, 0), (0, 1))):
                        for i in range(NDP):
                            nc.tensor.matmul(
                                aps,
                                qb[:, ti, 2 * i:2 * i + 2, qsl],
                                kt[:, tk, 2 * i:2 * i + 2, :],
                                start=(term == 0 and i == 0),
                                stop=(term == 2 and i == NDP - 1),
                                perf_mode=DR,
                            )
                    nc.vector.reduce_max(
                        M[:, qt, si:si + 1], aps, axis=AX.X, negate=True
                    )
                    nc.scalar.activation(
                        E[:, qt, k0:k0 + SEC], aps, AF.Exp,
                        bias=M[:, qt, si:si + 1], scale=1.0,
                        accum_out=S[:, qt, si:si + 1],
                    )

                def merge_et(qt):
                    """Softmax merge + E^T transpose/cast for one q-tile."""
                    negm = stats.tile([P, 1], F32, tag="negm", name=f"negm_{b}_{qt}")
                    nc.vector.tensor_reduce(
                        negm, M[:, qt, :], axis=AX.X, op=mybir.AluOpType.min
                    )
                    nc.scalar.activation(
                        F[:, qt, :], M[:, qt, :], AF.Exp, bias=negm, scale=-1.0
                    )
                    fs = stats.tile([P, NSEC], F32, tag="fs", name=f"fs_{b}_{qt}")
                    nc.vector.tensor_mul(fs, F[:, qt, :], S[:, qt, :])
                    sg = stats.tile([P, 1], F32, tag="sg", name=f"sg_{b}_{qt}")
                    nc.vector.reduce_sum(sg, fs, axis=AX.X)
                    nc.vector.reciprocal(R[:, qt:qt + 1], sg)
                    for si in range(NSEC):
                        sl = E[:, qt, si * SEC:(si + 1) * SEC]
                        nc.vector.tensor_scalar_mul(sl, sl, F[:, qt, si:si + 1])

                    # E^T via PE-array transposes (fp8), 8 blocks per PSUM
                    # bank, one ACT copy per bank
                    et8 = et8p.tile([P, LK // P, P], F8, tag="et8", name=f"et8_{b}_{qt}")
                    for half in range(2):
                        # fp8 PE transposes must write PSUM with element
                        # step 2; the ACT copy compacts the stride
                        ep = etps.tile([P, 8, P, 2], F8, tag="etps",
                                       name=f"etps_{b}_{qt}_{half}")
                        # one accumulation group per bank: start only on
                        # the first block so the bank's zero-region isn't
                        # re-armed over already-written blocks
                        for kc in range(8):
                            kcg = half * 8 + kc
                            nc.tensor.matmul(
                                ep[:, kc, :, 0], E[:, qt, kcg * P:(kcg + 1) * P],
                                ident, is_transpose=True,
                                start=(kc == 0), stop=(kc == 7),
                                skip_group_check=True,
                            )
                        nc.scalar.copy(out=et8[:, half * 8:(half + 1) * 8, :],
                                       in_=ep[:, :, :, 0])
                    return et8

                # ---- mm1 diagonal wavefront with woven supply ----
                kn_pending = []
                supply = []
                et8s = [None] * NQT
                for s in range(NSTEP):
                    blocks = [(si, s - si) for si in range(NSEC)
                              if 0 <= s - si < NQB]
                    # supply for step s+2 (and the tail Kn chunks), one
                    # piece between tiles
                    # Ordering rule: DRAM write->read deps come from
                    # program order, so a prepass piece must be issued
                    # before any transpose that reads it.  ppq(s+2) feeds
                    # this step's qp -> goes first; pp(K, s+3) feeds the
                    # NEXT step's kp -> rides after this step's kp/qp so
                    # transposes aren't queued behind it.
                    weave = []
                    if b == 0 and s + 2 < NQB:
                        r0 = (s + 2) * 256
                        weave.append(("ppq", r0))
                        weave.append(("ppq", r0 + 128))
                    if s + 2 < NSEC:
                        weave.append(("kp", s + 2, 0))
                        weave.append(("kp", s + 2, 1))
                    if s + 2 < NQB:
                        weave.append(("qp", 2 * (s + 2)))
                        weave.append(("qp", 2 * (s + 2) + 1))
                    if b == 0:
                        r0 = (s + 3) * 256
                        if r0 < lk:
                            weave.append(("pp", r0))
                            weave.append(("pp", r0 + 128))
                    if b == 1 and s < 4:
                        # second half of this batch's K prepass rides its own
                        # early steps (first half ran under b0's mm2)
                        r0 = (s + 4) * 256
                        weave.append(("pp", r0))
                        weave.append(("pp", r0 + 128))
                    if 6 <= s <= 9:
                        # kq16 kn chunk s-6 (slot of kt_{s-3}, freed after
                        # step s-3+3 = s)
                        for h in range(DH):
                            weave.append(("kn", s - 6, h))
                    if 7 <= s <= 10:
                        # u16 knC chunk 4+(s-7) (q-block s-7's slot)
                        for h in range(DH):
                            weave.append(("kn", s - 3, h))
                    supply.extend(weave)

                    def issue(op):
                        if op[0] == "kp":
                            kpath(op[1], op[2])
                        elif op[0] == "qp":
                            qpath(op[1])
                        elif op[0] == "pp":
                            prepass(b, k_dram, kf16, op[1], op[1] + 128)
                        elif op[0] == "ppq":
                            prepass(b, q_dram, qf16, op[1], op[1] + 128)
                        elif op[0] == "kn":
                            st = kn_dma(op[1], op[2])
                            kn_pending.append((op[1], op[2], 0, st, nc.vector))
                            kn_pending.append((op[1], op[2], 1, st, nc.vector))

                    tiles = [(si, 2 * qb + t) for si, qb in blocks
                             for t in range(2)]
                    for i, (si, qt) in enumerate(tiles):
                        mm1_tile(si, qt)
                        if s >= 7 and kn_pending and i % 2 == 0:
                            # drain kn lo-subs with priority in the tail so
                            # they don't pile up at the mm1->mm2 transition
                            kn_sub(*kn_pending.pop(0))
                        elif supply:
                            issue(supply.pop(0))
                        elif kn_pending:
                            kn_sub(*kn_pending.pop(0))
                    # flush the step's remaining supply in issue order --
                    # deferring a prepass past a transpose that reads it
                    # would break the program-order write->read dependency
                    while supply:
                        issue(supply.pop(0))
                    # early merges: q-block 0 has all its sections done
                    # after step 7, block 1 after step 8 -- build their E^T
                    # during the wavefront tail so mm2 is gated only by Kn
                    if s == NSTEP - 4:
                        et8s[0] = merge_et(0)
                    elif s == NSTEP - 3:
                        et8s[1] = merge_et(1)
                    elif s == NSTEP - 2:
                        et8s[2] = merge_et(2)

                # mm1 -> mm2 transition: leftover kn subs, then early
                # merges (qt0/qt1 merged in the mm1 tail weave below)
                for args in kn_pending:
                    kn_sub(*args)
                kn_pending = []
                if et8s[0] is None:
                    et8s[0] = merge_et(0)
                if et8s[1] is None:
                    et8s[1] = merge_et(1)

                # mm2 psum-group chunk order: by supply readiness
                CORDER = (0, 4, 1, 5, 2, 6, 3, 7)

                # ---- mm2 per q-tile (merges interleaved) ----
                for qt in range(NQT):
                    et8 = et8s[qt]
                    for dq in range(4):
                        DQ = d // 4
                        cps = psum.tile([P, DQ], F32, tag="ps",
                                        name=f"cps_{b}_{qt}_{dq}")
                        for j in range(DQ // 512):
                            dsl = slice(dq * DQ + j * 512, dq * DQ + (j + 1) * 512)
                            for ci, c in enumerate(CORDER):
                                for t in range(2):
                                    nc.tensor.matmul(
                                        cps[:, j * 512:(j + 1) * 512],
                                        et8[:, 2 * c:2 * c + 2, :],
                                        kn_tile(c)[:, t, :, dsl],
                                        start=(ci == 0 and t == 0),
                                        stop=(ci == NKP - 1 and t == 1),
                                        perf_mode=DR,
                                    )
                        ot = outp.tile([P, DQ], F16, tag="out",
                                       name=f"ot_{b}_{qt}_{dq}")
                        nc.scalar.mul(ot, cps, R[:, qt:qt + 1])
                        nc.sync.dma_start(
                            out=o_dram[b, qt * P:(qt + 1) * P,
                                       dq * DQ:(dq + 1) * DQ],
                            in_=ot,
                        )
                        # next batch's Q + half-K f32->f16 casts ride mm2's
                        # DMA slack (other K half rides b+1's own early mm1)
                        if b + 1 < b_per_core:
                            pi = qt * 4 + dq
                            if pi < 8:
                                prepass(b + 1, q_dram, qf16, pi * 128,
                                        (pi + 1) * 128)
                            elif pi < 16:
                                r0 = (pi - 8) * 128
                                prepass(b + 1, k_dram, kf16, r0, r0 + 128)
                    if qt + 2 < NQT and et8s[qt + 2] is None:
                        et8s[qt + 2] = merge_et(qt + 2)
                    if b + 1 < b_per_core and qt == 5:
                        # boundary pre-stage: next batch's first transpose
                        # pieces ride this batch's mm2 tail (stg is idle);
                        # only cast+sub remain at the batch boundary
                        bn = b + 1
                        staged[("q", 0)] = stg.tile(
                            [P, DC, P], F16, tag="stg", name=f"qstg_{bn}_0")
                        nc.sync.dma_start_transpose(
                            staged[("q", 0)], qf16[bn, 0:P, :])
                        staged[("k", 0, 0)] = stg.tile(
                            [P, DC, P], F16, tag="stg", name=f"kstg_{bn}_0")
                        nc.sync.dma_start_transpose(
                            staged[("k", 0, 0)], kf16[bn, 0:P, :])
                    if b + 1 < b_per_core and qt == 6:
                        bn = b + 1
                        staged[("q", 1)] = stg.tile(
                            [P, DC, P], F16, tag="stg", name=f"qstg_{bn}_1")
                        nc.sync.dma_start_transpose(
                            staged[("q", 1)], qf16[bn, P:2 * P, :])
                        staged[("k", 0, 1)] = stg.tile(
                            [P, DC, P], F16, tag="stg", name=f"kstg_{bn}_1")
                        nc.sync.dma_start_transpose(
                            staged[("k", 0, 1)], kf16[bn, P:2 * P, :])
    nc.compile()
    return nc


_PROGRAM = None


def _get_program():
    global _PROGRAM
    if _PROGRAM is None:
        _PROGRAM = build_program()
    return _PROGRAM


LAST_RESULTS = None  # BassKernelResults of the most recent kernel() call


def kernel(query: np.ndarray, key: np.ndarray) -> np.ndarray:
    global LAST_RESULTS
    query = np.ascontiguousarray(query, dtype=np.float32)
    key = np.ascontiguousarray(key, dtype=np.float32)
    assert query.shape == (B_FULL, LQ, D), query.shape
    assert key.shape == (B_FULL, LK, D), key.shape

    nc = _get_program()
    in_maps = [
        {
            "query": np.ascontiguousarray(query[i * B_PER_CORE:(i + 1) * B_PER_CORE]),
            "key": np.ascontiguousarray(key[i * B_PER_CORE:(i + 1) * B_PER_CORE]),
        }
        for i in range(N_CORES)
    ]
    res = run_bass_kernel_spmd(nc, in_maps, core_ids=list(range(N_CORES)))
    LAST_RESULTS = res
    out = np.concatenate([r["out"] for r in res.results], axis=0)
    return np.ascontiguousarray(out.astype(np.float32))


# revision 50
# speedup vs baseline: 1.2416x; 1.0210x over previous
"""DotAttention Trainium2 Bass kernel (fp8 DoubleRow edition).

out[b] = softmax(Q[b] @ K[b]^T, axis=-1) @ K[b]
  Q: [16, 1024, 4096] f32, K: [16, 2048, 4096] f32 -> out [16, 1024, 4096] f32

Sharding: batch dim across 8 NeuronCores (2 batches/core), fully local.

All matmuls run in fp8e4 (e4m3) DoubleRow mode: each instruction contracts
256 (two 128-blocks via the paired slot dim) at 0.5 cycles/row -- 4x the
fp16 streaming rate in the cost model.  Precision is held with error
feedback (hi+lo fp8 splits, lo = x - fp8(x) requantized):

  mm1 logits: A ~= Q8 K8^T + Qlo K8^T + Q8 Klo^T   (3 DR terms, f16-sourced)
  mm2:        C ~= E8 K8n + E8 Klon                (2 DR terms)

E is stored fp8 (exp writes e4m3 directly), rescaled in-place by the
online-softmax correction, and transposed per q-tile on the PE array
(fp8 transposes need stride-2 PSUM outputs; 8 blocks share one bank as
a single accumulation group so the bank's zero-region is armed once).
K gets two independent splits: transposed (for mm1: fp16 xbar transpose
-> ACT cast + DVE sub) and natural (for mm2: hi via SWDGE fp16->fp8
casting DMA straight from the fp16 scratch, lo via DVE sub against
streamed fp16 rows).

mm1 runs as a diagonal wavefront over (key-section, q-block) tiles with
supply (prepass casts / transposes / splits / mm2-K chunks) woven one
piece per tile, ordered so every DRAM write precedes its readers in
program order.  Sections rotate through 5 SBUF slots (2 steps of write
slack); freed slots stream in the mm2 K chunks during the wavefront
tail; merges for the first q-tiles run in the tail so mm2 starts gated
only on the last K chunk.  The next batch's prepass and first transpose
pieces ride this batch's mm2.

CPU-validated end-to-end numeric flow: rel err ~0.0123 (tolerance 2e-2);
cost-model time 843.8us/core vs 1047.7us fp16 baseline.
"""

import numpy as np

import concourse.bass as bass
import concourse.bacc as bacc
import concourse.mybir as mybir
import concourse.tile as tile
from concourse.bass_utils import run_bass_kernel_spmd

P = 128
N_CORES = 8
B_FULL, LQ, LK, D = 16, 1024, 2048, 4096
B_PER_CORE = B_FULL // N_CORES  # 2
DC = D // P                     # 32 d-chunks
NDP = DC // 2                   # 16 DoubleRow d-chunk pairs
NQT = LQ // P                   # 8 q-tiles
# online-softmax key sections
SEC = 256
NSEC = LK // SEC                # 8 sections
NKP = LK // 256                 # 8 key chunk-pairs for mm2
DH = 2
DHS = D // DH                   # 2048

F16 = mybir.dt.float16
F32 = mybir.dt.float32
F8 = mybir.dt.float8e4
AX = mybir.AxisListType
AF = mybir.ActivationFunctionType
DR = mybir.MatmulPerfMode.DoubleRow


def build_program(b_per_core=B_PER_CORE, lq=LQ, lk=LK, d=D):
    nc = bacc.Bacc("TRN2", target_bir_lowering=False, debug=False, num_swdge_queues=4)
    q_dram = nc.dram_tensor("query", [b_per_core, lq, d], F32, kind="ExternalInput").ap()
    k_dram = nc.dram_tensor("key", [b_per_core, lk, d], F32, kind="ExternalInput").ap()
    o_dram = nc.dram_tensor("out", [b_per_core, lq, d], F16, kind="ExternalOutput").ap()
    qf16 = nc.dram_tensor("qf16_scratch", [b_per_core, lq, d], F16, kind="Internal").ap()
    kf16 = nc.dram_tensor("kf16_scratch", [b_per_core, lk, d], F16, kind="Internal").ap()

    NQB = 4          # q-blocks of 2 q-tiles; mm1 runs a (section, q-block)
    NSTEP = NSEC + NQB - 1  # diagonal wavefront: block (si, qb) at step si+qb

    with tile.TileContext(nc) as tc:
        with (
            # 4x 16KB: mm1 Q^T hi+lo per q-block; mm2 Kn chunk-pairs 4..7
            tc.tile_pool(name="u16", bufs=4) as u16,
            # 5x 16KB: K^T section tiles (rotating, 2-step write slack);
            # mm2 Kn chunk-pairs 0..3
            tc.tile_pool(name="kq16", bufs=5) as kq16,
            # 16KB: E [P, NQT, LK] fp8 (exp writes e4m3 directly)
            tc.tile_pool(name="e16", bufs=1) as e16,
            # 8KB staging: f16 transpose pieces / f16 natural-K halves
            tc.tile_pool(name="stg", bufs=4) as stg,
            tc.tile_pool(name="et8p", bufs=3) as et8p,
            tc.tile_pool(name="outp", bufs=3) as outp,
            tc.tile_pool(name="stats", bufs=2) as stats,
            tc.tile_pool(name="psum", bufs=3, space="PSUM") as psum,
            tc.tile_pool(name="etps", bufs=2, space="PSUM") as etps,
        ):
            def prepass(b, src, dst, r0, r1):
                """fp32 -> fp16 cast via SWDGE DRAM->DRAM DMA (gpsimd queues)."""
                nc.gpsimd.dma_start(out=dst[b, r0:r1, :], in_=src[b, r0:r1, :])

            from concourse.masks import make_identity

            ident = None
            staged = {}

            for b in range(b_per_core):
                if b == 0:
                    prepass(b, q_dram, qf16, 0, 256)
                    prepass(b, k_dram, kf16, 0, 256)

                # Q^T hi+lo, one tile per q-block.  Slot of q-block c is
                # reused by mm2 chunk-pair 4+c; q-block c's last mm1 reader
                # is step 7+c, so the late K chunks stream in during the
                # wavefront tail instead of after mm1.
                qblks = [
                    u16.tile([P, 2, DC, 2 * P], F8, tag="u16", name=f"qb_{b}_{j}")
                    for j in range(NQB)
                ]
                knCs = [
                    u16.tile([P, 2, 2, d], F8, tag="u16", name=f"knC_{b}_{c}")
                    for c in range(4)
                ]
                # kq16: kt_si -> slot si%4; kn_j -> slot of kt_{4+j} (freed
                # after step 7+j).
                kts = [
                    kq16.tile([P, 2, DC, SEC], F8, tag="k16", name=f"kt_{b}_{si}")
                    for si in range(NSEC)
                ]
                kns = [
                    kq16.tile([P, 2, 2, d], F8, tag="k16", name=f"kn_{b}_{c}")
                    for c in range(4)
                ]

                # staged transpose pieces carried across the batch
                # boundary (filled during the previous batch's mm2)
                def qpath_dma(j):
                    r0 = j * P
                    s = stg.tile([P, DC, P], F16, tag="stg", name=f"qstg_{b}_{j}")
                    nc.sync.dma_start_transpose(s, qf16[b, r0:r0 + P, :])
                    return s

                def qpath_fin(j, s):
                    qb = qblks[j // 2]
                    c0 = (j % 2) * P
                    nc.scalar.copy(out=qb[:, 0, :, c0:c0 + P], in_=s)
                    nc.vector.tensor_sub(
                        out=qb[:, 1, :, c0:c0 + P], in0=s, in1=qb[:, 0, :, c0:c0 + P],
                    )

                def qpath(j):
                    """Transpose+split one 128-row piece of Q (q-tile j).
                    All subs stay on DVE: a slow Pool sub at the head of the
                    in-order Pool queue would block SWDGE descriptor
                    generation for every casting DMA behind it."""
                    qpath_fin(j, staged.pop(("q", j), None) or qpath_dma(j))

                def kpath_dma(si, j):
                    r0 = si * SEC + j * P
                    s = stg.tile([P, DC, P], F16, tag="stg", name=f"kstg_{b}_{r0}")
                    nc.sync.dma_start_transpose(s, kf16[b, r0:r0 + P, :])
                    return s

                def kpath_fin(si, j, s):
                    kt = kts[si]
                    c0 = j * P
                    nc.scalar.copy(out=kt[:, 0, :, c0:c0 + P], in_=s)
                    nc.vector.tensor_sub(
                        out=kt[:, 1, :, c0:c0 + P], in0=s, in1=kt[:, 0, :, c0:c0 + P],
                    )

                def kpath(si, j):
                    """Transpose+split one 128-key piece of K section si."""
                    kpath_fin(si, j, staged.pop(("k", si, j), None) or kpath_dma(si, j))

                def kn_tile(c):
                    return kns[c] if c < 4 else knCs[c - 4]

                def kn_dma(c, h):
                    """One d-half of mm2 K chunk-pair c: f16 rows to staging +
                    hi tensor via casting SWDGE DMA (block-paired rows)."""
                    kn = kn_tile(c)
                    src = kf16[b, 256 * c:256 * (c + 1),
                               h * DHS:(h + 1) * DHS].rearrange(
                        "(two p) d -> p two d", two=2)
                    s = stg.tile([P, 2, DHS], F16, tag="stg",
                                 name=f"knstg_{b}_{c}_{h}")
                    nc.sync.dma_start(out=s, in_=src)
                    nc.gpsimd.dma_start(
                        out=kn[:, 0, :, h * DHS:(h + 1) * DHS], in_=src)
                    return s

                def kn_sub(c, h, q, s, eng):
                    """DVE/Pool lo = f16 - hi for one d-quarter piece."""
                    d0 = h * DHS + q * (DHS // 2)
                    kn = kn_tile(c)
                    eng.tensor_sub(
                        out=kn[:, 1, :, d0:d0 + DHS // 2],
                        in0=s[:, :, q * (DHS // 2):(q + 1) * (DHS // 2)],
                        in1=kn[:, 0, :, d0:d0 + DHS // 2],
                    )

                # stats
                M = stats.tile([P, NQT, NSEC], F32, tag="m", name=f"M_{b}")
                S = stats.tile([P, NQT, NSEC], F32, tag="s", name=f"S_{b}")
                F = stats.tile([P, NQT, NSEC], F32, tag="f", name=f"F_{b}")
                R = stats.tile([P, NQT], F32, tag="r", name=f"R_{b}")
                E = e16.tile([P, NQT, lk], F8, tag="e", name=f"E_{b}")

                # ---- stage-1 lead-in: step-0 operands ----
                kpath(0, 0)
                qpath(0)
                if ident is None:
                    ident = stats.tile([P, P], F8, tag="ident", name="ident")
                    make_identity(nc, ident)
                kpath(0, 1)
                qpath(1)
                if b == 0:
                    prepass(b, q_dram, qf16, 256, 512)
                    prepass(b, k_dram, kf16, 256, 512)
                kpath(1, 0)
                if b == 0:
                    prepass(b, k_dram, kf16, 512, 768)
                kpath(1, 1)
                qpath(2)
                qpath(3)
                staged[("k", 2, 0)] = kpath_dma(2, 0)
                if b == 0:
                    prepass(b, k_dram, kf16, 768, 1024)
                staged[("k", 2, 1)] = kpath_dma(2, 1)

                def mm1_tile(si, qt):
                    kt = kts[si]
                    k0 = si * SEC
                    aps = psum.tile([P, SEC], F32, tag="ps",
                                    name=f"aps_{b}_{si}_{qt}")
                    qb = qblks[qt // 2]
                    qsl = slice((qt % 2) * P, (qt % 2) * P + P)
                    for term, (ti, tk) in enumerate(((0, 0), (1, 0), (0, 1))):
                        for i in range(NDP):
                            nc.tensor.matmul(
                                aps,
                                qb[:, ti, 2 * i:2 * i + 2, qsl],
                                kt[:, tk, 2 * i:2 * i + 2, :],
                                start=(term == 0 and i == 0),
                                stop=(term == 2 and i == NDP - 1),
                                perf_mode=DR,
                            )
                    nc.vector.reduce_max(
                        M[:, qt, si:si + 1], aps, axis=AX.X, negate=True
                    )
                    nc.scalar.activation(
                        E[:, qt, k0:k0 + SEC], aps, AF.Exp,
                        bias=M[:, qt, si:si + 1], scale=1.0,
                        accum_out=S[:, qt, si:si + 1],
                    )

                def merge_et(qt):
                    """Softmax merge + E^T transpose/cast for one q-tile."""
                    negm = stats.tile([P, 1], F32, tag="negm", name=f"negm_{b}_{qt}")
                    nc.vector.tensor_reduce(
                        negm, M[:, qt, :], axis=AX.X, op=mybir.AluOpType.min
                    )
                    nc.scalar.activation(
                        F[:, qt, :], M[:, qt, :], AF.Exp, bias=negm, scale=-1.0
                    )
                    fs = stats.tile([P, NSEC], F32, tag="fs", name=f"fs_{b}_{qt}")
                    nc.vector.tensor_mul(fs, F[:, qt, :], S[:, qt, :])
                    sg = stats.tile([P, 1], F32, tag="sg", name=f"sg_{b}_{qt}")
                    nc.vector.reduce_sum(sg, fs, axis=AX.X)
                    nc.vector.reciprocal(R[:, qt:qt + 1], sg)
                    for si in range(NSEC):
                        sl = E[:, qt, si * SEC:(si + 1) * SEC]
                        nc.vector.tensor_scalar_mul(sl, sl, F[:, qt, si:si + 1])

                    # E^T via PE-array transposes (fp8), 8 blocks per PSUM
                    # bank, one ACT copy per bank
                    et8 = et8p.tile([P, LK // P, P], F8, tag="et8", name=f"et8_{b}_{qt}")
                    for half in range(2):
                        # fp8 PE transposes must write PSUM with element
                        # step 2; the ACT copy compacts the stride
                        ep = etps.tile([P, 8, P, 2], F8, tag="etps",
                                       name=f"etps_{b}_{qt}_{half}")
                        # one accumulation group per bank: start only on
                        # the first block so the bank's zero-region isn't
                        # re-armed over already-written blocks
                        for kc in range(8):
                            kcg = half * 8 + kc
                            nc.tensor.matmul(
                                ep[:, kc, :, 0], E[:, qt, kcg * P:(kcg + 1) * P],
                                ident, is_transpose=True,
                                start=(kc == 0), stop=(kc == 7),
                                skip_group_check=True,
                            )
                        nc.scalar.copy(out=et8[:, half * 8:(half + 1) * 8, :],
                                       in_=ep[:, :, :, 0])
                    return et8

                # ---- mm1 diagonal wavefront with woven supply ----
                kn_pending = []
                supply = []
                et8s = [None] * NQT
                for s in range(NSTEP):
                    blocks = [(si, s - si) for si in range(NSEC)
                              if 0 <= s - si < NQB]
                    # supply for step s+2 (and the tail Kn chunks), one
                    # piece between tiles
                    # Ordering rule: DRAM write->read deps come from
                    # program order, so a prepass piece must be issued
                    # before any transpose that reads it.  ppq(s+2) feeds
                    # this step's qp -> goes first; pp(K, s+3) feeds the
                    # NEXT step's kp -> rides after this step's kp/qp so
                    # transposes aren't queued behind it.
                    weave = []
                    if b == 0 and s + 2 < NQB:
                        r0 = (s + 2) * 256
                        weave.append(("ppq", r0))
                        weave.append(("ppq", r0 + 128))
                    if s + 3 < NSEC:
                        # transposes one step ahead of their cast+sub: they
                        # only need a stg slot, not the WAR-gated kt slot
                        weave.append(("kpT", s + 3, 0))
                        weave.append(("kpT", s + 3, 1))
                    if s + 2 < NSEC:
                        weave.append(("kpF", s + 2, 0))
                        weave.append(("kpF", s + 2, 1))
                    if s + 2 < NQB:
                        weave.append(("qp", 2 * (s + 2)))
                        weave.append(("qp", 2 * (s + 2) + 1))
                    if b == 0:
                        # one step ahead of that section's kpT transposes
                        r0 = (s + 4) * 256
                        if r0 < lk:
                            weave.append(("pp", r0))
                            weave.append(("pp", r0 + 128))
                    if b == 1 and s < 4:
                        # second half of this batch's K prepass rides its own
                        # early steps (first half ran under b0's mm2)
                        r0 = (s + 4) * 256
                        weave.append(("pp", r0))
                        weave.append(("pp", r0 + 128))
                    if 6 <= s <= 9:
                        # kq16 kn chunk s-6 (slot of kt_{s-3}, freed after
                        # step s-3+3 = s)
                        for h in range(DH):
                            weave.append(("kn", s - 6, h))
                    if 7 <= s <= 10:
                        # u16 knC chunk 4+(s-7) (q-block s-7's slot)
                        for h in range(DH):
                            weave.append(("kn", s - 3, h))
                    supply.extend(weave)

                    def issue(op):
                        if op[0] == "kpT":
                            staged[("k", op[1], op[2])] = kpath_dma(op[1], op[2])
                        elif op[0] == "kpF":
                            kpath(op[1], op[2])
                        elif op[0] == "kp":
                            kpath(op[1], op[2])
                        elif op[0] == "qp":
                            qpath(op[1])
                        elif op[0] == "pp":
                            prepass(b, k_dram, kf16, op[1], op[1] + 128)
                        elif op[0] == "ppq":
                            prepass(b, q_dram, qf16, op[1], op[1] + 128)
                        elif op[0] == "kn":
                            st = kn_dma(op[1], op[2])
                            kn_pending.append((op[1], op[2], 0, st, nc.vector))
                            kn_pending.append((op[1], op[2], 1, st, nc.vector))

                    tiles = [(si, 2 * qb + t) for si, qb in blocks
                             for t in range(2)]
                    for i, (si, qt) in enumerate(tiles):
                        mm1_tile(si, qt)
                        if s >= 7 and kn_pending and i % 2 == 0:
                            # drain kn lo-subs with priority in the tail so
                            # they don't pile up at the mm1->mm2 transition
                            kn_sub(*kn_pending.pop(0))
                        elif supply:
                            issue(supply.pop(0))
                        elif kn_pending:
                            kn_sub(*kn_pending.pop(0))
                    # flush the step's remaining supply in issue order --
                    # deferring a prepass past a transpose that reads it
                    # would break the program-order write->read dependency
                    while supply:
                        issue(supply.pop(0))
                    # early merges: q-block 0 has all its sections done
                    # after step 7, block 1 after step 8 -- build their E^T
                    # during the wavefront tail so mm2 is gated only by Kn
                    if s == NSTEP - 4:
                        et8s[0] = merge_et(0)
                    elif s == NSTEP - 3:
                        et8s[1] = merge_et(1)
                    elif s == NSTEP - 2:
                        et8s[2] = merge_et(2)

                # mm1 -> mm2 transition: leftover kn subs, then early
                # merges (qt0/qt1 merged in the mm1 tail weave below)
                for args in kn_pending:
                    kn_sub(*args)
                kn_pending = []
                if et8s[0] is None:
                    et8s[0] = merge_et(0)
                if et8s[1] is None:
                    et8s[1] = merge_et(1)

                # mm2 psum-group chunk order: by supply readiness
                CORDER = (0, 4, 1, 5, 2, 6, 3, 7)

                # ---- mm2 per q-tile (merges interleaved) ----
                for qt in range(NQT):
                    et8 = et8s[qt]
                    for dq in range(4):
                        DQ = d // 4
                        cps = psum.tile([P, DQ], F32, tag="ps",
                                        name=f"cps_{b}_{qt}_{dq}")
                        for j in range(DQ // 512):
                            dsl = slice(dq * DQ + j * 512, dq * DQ + (j + 1) * 512)
                            for ci, c in enumerate(CORDER):
                                for t in range(2):
                                    nc.tensor.matmul(
                                        cps[:, j * 512:(j + 1) * 512],
                                        et8[:, 2 * c:2 * c + 2, :],
                                        kn_tile(c)[:, t, :, dsl],
                                        start=(ci == 0 and t == 0),
                                        stop=(ci == NKP - 1 and t == 1),
                                        perf_mode=DR,
                                    )
                        ot = outp.tile([P, DQ], F16, tag="out",
                                       name=f"ot_{b}_{qt}_{dq}")
                        nc.scalar.mul(ot, cps, R[:, qt:qt + 1])
                        nc.sync.dma_start(
                            out=o_dram[b, qt * P:(qt + 1) * P,
                                       dq * DQ:(dq + 1) * DQ],
                            in_=ot,
                        )
                        # next batch's Q + half-K f32->f16 casts ride mm2's
                        # DMA slack (other K half rides b+1's own early mm1)
                        if b + 1 < b_per_core:
                            pi = qt * 4 + dq
                            if pi < 8:
                                prepass(b + 1, q_dram, qf16, pi * 128,
                                        (pi + 1) * 128)
                            elif pi < 16:
                                r0 = (pi - 8) * 128
                                prepass(b + 1, k_dram, kf16, r0, r0 + 128)
                    if qt + 2 < NQT and et8s[qt + 2] is None:
                        et8s[qt + 2] = merge_et(qt + 2)
                    if b + 1 < b_per_core and qt == 5:
                        # boundary pre-stage: next batch's first transpose
                        # pieces ride this batch's mm2 tail (stg is idle);
                        # only cast+sub remain at the batch boundary
                        bn = b + 1
                        staged[("q", 0)] = stg.tile(
                            [P, DC, P], F16, tag="stg", name=f"qstg_{bn}_0")
                        nc.sync.dma_start_transpose(
                            staged[("q", 0)], qf16[bn, 0:P, :])
                        staged[("k", 0, 0)] = stg.tile(
                            [P, DC, P], F16, tag="stg", name=f"kstg_{bn}_0")
                        nc.sync.dma_start_transpose(
                            staged[("k", 0, 0)], kf16[bn, 0:P, :])
                    if b + 1 < b_per_core and qt == 6:
                        bn = b + 1
                        staged[("q", 1)] = stg.tile(
                            [P, DC, P], F16, tag="stg", name=f"qstg_{bn}_1")
                        nc.sync.dma_start_transpose(
                            staged[("q", 1)], qf16[bn, P:2 * P, :])
                        staged[("k", 0, 1)] = stg.tile(
                            [P, DC, P], F16, tag="stg", name=f"kstg_{bn}_1")
                        nc.sync.dma_start_transpose(
                            staged[("k", 0, 1)], kf16[bn, P:2 * P, :])
    nc.compile()
    return nc


_PROGRAM = None


def _get_program():
    global _PROGRAM
    if _PROGRAM is None:
        _PROGRAM = build_program()
    return _PROGRAM


LAST_RESULTS = None  # BassKernelResults of the most recent kernel() call


def kernel(query: np.ndarray, key: np.ndarray) -> np.ndarray:
    global LAST_RESULTS
    query = np.ascontiguousarray(query, dtype=np.float32)
    key = np.ascontiguousarray(key, dtype=np.float32)
    assert query.shape == (B_FULL, LQ, D), query.shape
    assert key.shape == (B_FULL, LK, D), key.shape

    nc = _get_program()
    in_maps = [
        {
            "query": np.ascontiguousarray(query[i * B_PER_CORE:(i + 1) * B_PER_CORE]),
            "key": np.ascontiguousarray(key[i * B_PER_CORE:(i + 1) * B_PER_CORE]),
        }
        for i in range(N_CORES)
    ]
    res = run_bass_kernel_spmd(nc, in_maps, core_ids=list(range(N_CORES)))
    LAST_RESULTS = res
    out = np.concatenate([r["out"] for r in res.results], axis=0)
    return np.ascontiguousarray(out.astype(np.float32))
